# revision 60
# baseline (speedup 1.0000x reference)
"""Causal multi-head attention (B=4, T=2048, C=1024, H=16) on 8 TRN2 cores.

Sharding: core c = (batch b = c // 2, head-group g = c % 2). Each core runs
QKV projection + attention + its half of the output projection for one
batch and 8 heads (Megatron-style column-parallel wqkv / row-parallel wo);
the two partial wo products per batch are summed on the host during
unsharding.

Device layouts (per core):
  xT    [C, T]        x[b] transposed (C on partitions for QKV matmuls)
  wqkvT [C, 3*512]    head-group slice of wqkv, transposed; cols [Q|K|V]
  woT   [512, C]      wo columns for this head-group, transposed
  outT  [C, T]        partial (x @ wqkv.T -> attn -> @ wo.T).T

All matmuls run in float32r (tf32-like: ~1.5e-4 rel err, ~3.3x faster
than fp32 on the PE). Scores are computed transposed (k on partitions) so
softmax P.T feeds the AV matmul directly with no transposes; the softmax
denominator rides along as a 65th row of V (column of ones), and the
causal mask is an affine_select on the diagonal score tiles. exp() is
applied without max-subtraction: scores are ~N(0,1), far inside fp32 exp
range.

Scheduling: the T dimension is processed in four 512-chunks. Attention on
chunk ch is emitted with the (PE-only) QKV projection of chunk ch+1 and
the WO projection of chunk ch-1 round-robin-interleaved into its
(head-pair, k-tile) loop, so the PE array keeps streaming while ACT runs
the softmax exps — that keeps the HAM clock gate at 2.4 GHz. Heads are
processed in pairs living at partition offsets 0/64: their K=64 score
matmuls target disjoint row-halves of the PE array and run concurrently.

Edges: a few dependency-free warm-up matmuls run during the input-DMA
fill so the HAM clock gate is already at 2.4 GHz when the first QKV
chain's data lands. Each pair's normalization leaves PSUM in one
full-width copy per pos bank (pos recycles after a single DVE op, so the
next pair's AV matmuls aren't gated ~2.7us behind four split copies) and
its recips/muls are paced one-per-step into the next pair's loop so they
don't monopolize the in-order DVE queue. A few fill units are pre-popped
ahead of each pair's first score matmul, which otherwise head-of-line
blocks ready work in the in-order PE queue. All WO projections fill the
last attention phase. The tail: the last chunk's WO chains accumulate
kt0-1 during the final pair's loop (partial straight to outT), kt2
streams during the final normalization (keeping the clock gate warm),
and only kt3 waits for it, stored to outT2 for the host to add back.
That final normalization swaps denominator lanes with PE permutation
matmuls (swapC) instead of ~1.7us SBUF->SBUF DMAs. outT is bf16 (host
sums partials in f32), halving store traffic; stores alternate between
the sync and scalar DMA queues (never gpsimd, which carries the
latency-critical normalization lane shifts).
"""

import sys

sys.path.insert(0, "/opt/trn_rl_repo")

import numpy as np

import concourse.bass as bass  # noqa: F401  (AP helpers)
import concourse.bass_utils as _bu
import concourse.mybir as mybir
import concourse.tile as tile
from concourse import bacc
from concourse.bass_utils import run_bass_kernel_spmd

# NOTE: walrus's --enable-ldw-opt=true (used by an earlier fp32r version of
# this kernel) rejects bf16 LDWEIGHTS ("InstLdweights is not compatible with
# LDW optimization"), so this kernel runs with the concourse default (off).

F32 = mybir.dt.float32
F32R = mybir.dt.float32r
BF16 = mybir.dt.bfloat16
EXP = mybir.ActivationFunctionType.Exp

B, T, C, H = 4, 2048, 1024, 16
HD = 64  # head dim
HPC = 8  # heads per core
GF = HPC * HD  # 512 group features
CHUNK = 512
NCHUNK = T // CHUNK  # 4
KT_C = C // 128  # 8 k-tiles over C
SCALE = 1.0 / np.sqrt(HD)

_CACHE = {}


def _build():
    nc = bacc.Bacc(
        "TRN2", target_bir_lowering=False, debug=False, num_devices=8
    )
    xT = nc.dram_tensor("xT", [C, T], BF16, kind="ExternalInput")
    wqkvT = nc.dram_tensor("wqkvT", [C, 3 * GF], BF16, kind="ExternalInput")
    woT = nc.dram_tensor("woT", [GF, C], BF16, kind="ExternalInput")
    # 64-lane swap permutation (host-built): swapC.T @ x swaps the two
    # partition halves of x on the PE — used by the final normalization
    # instead of a ~1.7us SBUF->SBUF lane-shift DMA
    swapC = nc.dram_tensor("swapC", [128, 128], F32, kind="ExternalInput")
    # bf16 output halves the store traffic (the two per-batch partials are
    # summed on the host in f32; the extra rounding is ~3e-3 rel, well
    # inside the 2e-2 budget)
    outT = nc.dram_tensor("outT", [C, T], BF16, kind="ExternalOutput")
    # kt3 (last head-pair) contribution of the last chunk's WO, stored
    # separately so the chunk-3 WO partials can stream out before the final
    # normalization; host adds it onto outT[:, 3*CHUNK:].
    outT2 = nc.dram_tensor("outT2", [C, CHUNK], BF16, kind="ExternalOutput")

    xT_re = xT.ap().rearrange("(kt p) t -> p kt t", p=128)
    wq_re = wqkvT.ap().rearrange("(kt p) f -> p kt f", p=128)
    wo_re = woT.ap().rearrange("(kt p) f -> p kt f", p=128)
    outT2_re = outT2.ap().rearrange("(g two p) t -> p g two t", p=128, two=2)

    with tile.TileContext(nc) as tc:
        with (
            tc.tile_pool(name="weights", bufs=1) as wp,
            tc.tile_pool(name="persist", bufs=1) as persist,
            tc.tile_pool(name="xin", bufs=2) as xp,
            tc.tile_pool(name="qt", bufs=2) as qtp,
            tc.tile_pool(name="pt", bufs=2) as ptp,
            tc.tile_pool(name="yt", bufs=4) as ytp,
            tc.tile_pool(name="small", bufs=2) as smp,
            tc.tile_pool(name="ost", bufs=4) as ostp,
            tc.tile_pool(name="ost2", bufs=4) as ost2p,
            tc.tile_pool(name="proj", bufs=2, space="PSUM") as projp,
            tc.tile_pool(name="sps", bufs=2, space="PSUM") as spsp,
            tc.tile_pool(name="pops", bufs=2, space="PSUM") as pop,
        ):
            # wq split column-wise: [Q four 128-col tiles][K four][V one 512-col]
            wqq = [[wp.tile([128, 128], BF16, name=f"wqq{kt}_{g}") for g in range(4)]
                   for kt in range(KT_C)]
            wqk = [[wp.tile([128, 128], BF16, name=f"wqk{kt}_{g}") for g in range(4)]
                   for kt in range(KT_C)]
            wqv = [wp.tile([128, GF], BF16, name=f"wqv{kt}") for kt in range(KT_C)]
            wo_sb = wp.tile([128, GF // 128, C], BF16)

            # per-chunk persistent K.T / V_aug. V is padded to 128 columns
            # with a 64-wide all-ones block: the AV matmul then emits the
            # softmax denominator replicated across 64 PSUM partitions, so
            # normalization needs no gpsimd partition_broadcast (no gpsimd
            # library thrash). Even heads are [V | ones] (y on lanes 0-63),
            # odd heads [ones | V] (y on lanes 64-127, where ytc wants it) —
            # DVE ops must be lane-aligned on HW, and the custom reciprocal
            # only works at partition base 0, so each head needs exactly one
            # DMA lane-shift of 1/d (resp. d) across the 64-lane boundary.
            KTc = [persist.tile([128, 4, CHUNK], BF16, name=f"KT{i}") for i in range(NCHUNK)]
            VAc = [persist.tile([128, 4, HPC, 2 * HD], BF16, name=f"VA{i}") for i in range(NCHUNK)]

            # HAM pre-warm: a few dependency-free matmuls on a memset tile
            # keep the PE busy through the input-DMA fill, so the 4096-cycle
            # activity window flips the clock gate to 2.4 GHz right as the
            # first QKV chain's data lands (otherwise the first ~3.4us of
            # real matmuls run at 1.2 GHz). Wide 512-col dummies: enough
            # cold-clock runtime to warm the gate without clogging the
            # in-order PE queue past the point where real data arrives.
            warm = wp.tile([128, CHUNK], BF16, name="warm")
            swap_sb = wp.tile([128, 128], F32, name="swap")
            swap_bf = wp.tile([128, 128], BF16, name="swapbf")
            nc.vector.memset(warm[:], 0.25)
            warm_ps = spsp.tile([128, 2 * CHUNK], F32, tag="sps", name="warmps")
            for _ in range(7):
                nc.tensor.matmul(
                    warm_ps[:, 0:CHUNK], warm[:, 0:128], warm[:], start=True, stop=True
                )

            for i in range(NCHUNK):
                for h in range(HPC):
                    lo = HD if h % 2 == 0 else 0
                    nc.vector.memset(VAc[i][:, :, h, lo : lo + HD], 1.0)

            xts = {}
            QTcs = {}
            ytcs = {}

            def load_x(ch):
                xts[ch] = [
                    xp.tile([128, CHUNK], BF16, tag="xt", bufs=24, name=f"xt{ch}_{kt}")
                    for kt in range(KT_C)
                ]
                for kt in range(KT_C):
                    nc.sync.dma_start(
                        xts[ch][kt][:],
                        xT_re[:, kt, ch * CHUNK : (ch + 1) * CHUNK],
                    )

            def qkv_thunks(ch, spread=False):
                """Projection work for chunk ch as ~108 single-instruction
                units (8 matmuls + 1 copy per chain, 12 chains) so fills can
                slot between dependent attention steps at matmul granularity."""
                QTcs[ch] = qtp.tile([128, 4, CHUNK], BF16, tag="qtc", name=f"qtc{ch}")
                units = []
                seq = [0]

                def chain_psum(st):
                    if not spread:
                        return projp.tile([128, CHUNK], F32, tag="proj", name="ps")
                    i = seq[0]
                    seq[0] += 1
                    # spread over projp+pop only: the spsp slots must only
                    # ever hold uniform [128, 2*CHUNK] sps2 tiles, else the
                    # pool's extent-based reuse deps miss orderings
                    pool, tag = [
                        (projp, "proj"), (projp, "proj"), (pop, "po"), (pop, "po"),
                    ][i % 4]
                    return pool.tile([128, CHUNK], F32, tag=tag, name=f"pps{ch}_{i}")

                def qk_units(m):
                    st = {}

                    def mm(kt, m=m, st=st):
                        if kt == 0:
                            st["ps"] = chain_psum(st)
                        wt = wqq[kt][m] if m < 4 else wqk[kt][m - 4]
                        nc.tensor.matmul(
                            st["ps"][:],
                            wt[:],
                            xts[ch][kt][:],
                            start=(kt == 0),
                            stop=(kt == KT_C - 1),
                        )

                    def cp(m=m, st=st):
                        dst = QTcs[ch][:, m] if m < 4 else KTc[ch][:, m - 4]
                        nc.vector.tensor_copy(dst, st["ps"][:])

                    return [lambda kt=kt: mm(kt) for kt in range(KT_C)] + [cp]

                def v_units(t4):
                    st = {}

                    def mm(kt, t4=t4, st=st):
                        if kt == 0:
                            st["ps"] = chain_psum(st)
                        nc.tensor.matmul(
                            st["ps"][:],
                            xts[ch][kt][:, t4 * 128 : (t4 + 1) * 128],
                            wqv[kt][:],
                            start=(kt == 0),
                            stop=(kt == KT_C - 1),
                        )

                    def cp(t4=t4, st=st):
                        src = st["ps"][:].rearrange(
                            "p (h4 two d) -> p two h4 d", two=2, d=HD
                        )
                        dst = VAc[ch][:, t4].rearrange(
                            "p (h4 two) c -> p two h4 c", two=2
                        )
                        nc.vector.tensor_copy(dst[:, 0, :, 0:HD], src[:, 0])
                        nc.vector.tensor_copy(dst[:, 1, :, HD : 2 * HD], src[:, 1])

                    return [lambda kt=kt: mm(kt) for kt in range(KT_C)] + [cp]

                if spread:
                    # prologue (chunk 0): kt-major pairwise emission for the
                    # leading chains, matching the x-tile DMA arrival order —
                    # chain-major would park the in-order PE queue on chain
                    # 0's last tiles while later chains' early tiles (already
                    # in SBUF) sit ready behind it
                    uq0, uk0 = qk_units(0), qk_units(4)
                    uv = [v_units(t4) for t4 in range(4)]
                    units += [u for p in zip(uq0, uk0) for u in p]
                    units += [u for p in zip(uv[0], uv[1]) for u in p]
                    units += [u for p in zip(uv[2], uv[3]) for u in p]
                else:
                    units += qk_units(0)
                    units += qk_units(4)
                    for t4 in range(4):
                        units += v_units(t4)
                for m in (1, 5, 2, 6, 3, 7):
                    units += qk_units(m)
                return units

            # store DMAs alternate between the sync and scalar queues so
            # back-to-back stores don't serialize on one engine's ~600ns
            # descriptor-gen time. The gpsimd queue is deliberately excluded:
            # it carries the normalization lane-shift DMAs, whose latency is
            # on the critical path and must not sit behind 128KB store
            # transfers.
            _st_engs = [nc.sync, nc.scalar]
            _stq = [0]

            def _store(dst, src):
                _st_engs[_stq[0] % 2].dma_start(dst, src)
                _stq[0] += 1

            def wo_chains(ch):
                """Output projection for chunk ch as 8 chains of 5
                single-instruction units (4 matmuls + 1 copy/dma)."""
                cs = slice(ch * CHUNK, (ch + 1) * CHUNK)

                def wo_units(m):
                    st = {}

                    def mm(kt, m=m, st=st):
                        if kt == 0:
                            st["ps"] = projp.tile(
                                [128, CHUNK], F32, tag="proj", name=f"wop{ch}_{m}"
                            )
                        nc.tensor.matmul(
                            st["ps"][:],
                            wo_sb[:, kt, m * 128 : (m + 1) * 128],
                            ytcs[ch][:, kt],
                            start=(kt == 0),
                            stop=(kt == GF // 128 - 1),
                        )

                    def out(m=m, st=st):
                        ot = ostp.tile([128, CHUNK], BF16)
                        nc.vector.tensor_copy(ot[:], st["ps"][:])
                        _store(outT[m * 128 : (m + 1) * 128, cs], ot[:])

                    return [lambda kt=kt: mm(kt) for kt in range(GF // 128)] + [out]

                return [wo_units(m) for m in range(8)]

            def wo_tail_units():
                """Last chunk's WO, split so the final normalization gates
                little PE work: `early` (drained during the last head-pair's
                kt loop; depends only on pairs 0-1, normalized long before)
                accumulates kt0-1 of every chain and stores that partial
                straight to outT; `late` (after pair 2/3 normalize) runs the
                kt2+kt3 matmuls, stored to outT2 for the host to add back."""
                ch = NCHUNK - 1
                cs = slice(ch * CHUNK, (ch + 1) * CHUNK)
                early, late = [], []
                for m in range(8):
                    st = {}

                    def mm01(kt, m=m, st=st):
                        if kt == 0:
                            st["ps"] = projp.tile(
                                [128, CHUNK], F32, tag="proj", name=f"wop3e_{m}"
                            )
                        nc.tensor.matmul(
                            st["ps"][:],
                            wo_sb[:, kt, m * 128 : (m + 1) * 128],
                            ytcs[ch][:, kt],
                            start=(kt == 0),
                            stop=(kt == 1),
                        )

                    def outp(m=m, st=st):
                        ot = ostp.tile([128, CHUNK], BF16)
                        # alternate DVE/ACT: during the last pair's loop the
                        # DVE is also running the previous pair's norm ops,
                        # and a copy stuck behind them would stall the
                        # 2-slot psum wave rotation (and with it the PE)
                        cp = nc.vector.tensor_copy if m % 2 == 0 else nc.scalar.copy
                        cp(ot[:], st["ps"][:])
                        _store(outT[m * 128 : (m + 1) * 128, cs], ot[:])

                    early += [lambda kt=kt, mm01=mm01: mm01(kt) for kt in range(2)]
                    early.append(outp)
                # Every late chain gets its own dedicated PSUM region — the
                # spsp banks (free once the last score/exp retires), the pos
                # banks (free once the stage-out copies run) and the projp
                # slots. That way all eight kt2 matmuls (which only need pair
                # 2) stream during the final normalization, keeping the HAM
                # clock gate warm, and only the kt3s wait for it.
                shared = {}

                def region(c, shared=shared):
                    if c < 4:
                        i = c // 2
                        if f"sps{i}" not in shared:
                            shared[f"sps{i}"] = spsp.tile(
                                [128, 2 * CHUNK], F32, tag="sps", name=f"wot{i}"
                            )
                        return shared[f"sps{i}"][:, (c % 2) * CHUNK : (c % 2 + 1) * CHUNK]
                    if c < 6:
                        k = f"po{c}"
                        if k not in shared:
                            shared[k] = pop.tile([128, CHUNK], F32, tag="po", name=k)
                        return shared[k][:]
                    k = f"pj{c}"
                    if k not in shared:
                        shared[k] = projp.tile([128, CHUNK], F32, tag="proj", name=k)
                    return shared[k][:]

                def mk_mm(m, kt):
                    def mm():
                        nc.tensor.matmul(
                            region(m),
                            wo_sb[:, kt, m * 128 : (m + 1) * 128],
                            ytcs[ch][:, kt],
                            start=(kt == 2),
                            stop=(kt == 3),
                        )

                    return mm

                def mk_out(m):
                    # chains pair up into one [128, 2*CHUNK] staging tile and
                    # one store, halving the tail's DMA issue serialization;
                    # the two copies ride different engines
                    def out3():
                        if f"ot{m // 2}" not in shared:
                            shared[f"ot{m // 2}"] = ost2p.tile(
                                [128, 2 * CHUNK], BF16, name=f"ot{m // 2}"
                            )
                        ot = shared[f"ot{m // 2}"]
                        cp = nc.scalar.copy if m % 2 == 0 else nc.vector.tensor_copy
                        cp(ot[:, (m % 2) * CHUNK : (m % 2 + 1) * CHUNK], region(m))
                        if m % 2 == 1:
                            _store(
                                outT2_re[:, m // 2], ot[:].rearrange(
                                    "p (two t) -> p two t", two=2
                                )
                            )

                    return out3

                late += [mk_mm(m, 2) for m in range(8)]
                for m in range(8):
                    late += [mk_mm(m, 3), mk_out(m)]
                return early, late

            def attention(qc, fill, pair3_fill=None):
                """Attention for q-chunk qc. The AV matmul for k-tile kt is
                emitted one step late (after the kt+1 score matmuls and any
                fill units), so the PE never sits behind ACT's exp in the
                in-order engine queue; `fill` units drain proportionally.
                `pair3_fill` units (which depend on pairs 0-2 being done)
                drain during the last pair's k-tile loop."""
                ytcs[qc] = ytp.tile([128, GF // 128, CHUNK], BF16, tag="ytc", name=f"ytc{qc}")
                ytc = ytcs[qc]
                QTc = QTcs[qc]
                last_chunk = qc == NCHUNK - 1
                nkt = (qc + 1) * 4
                nsteps = 4 * nkt
                total_fill = len(fill)
                done = [0]

                # for the last chunk, stretch the w-fill drain 2 steps into
                # pair 3 so the tail-chain drain takes over seamlessly
                drain_steps = (3 * nkt + 2) if last_chunk else nsteps
                p3_total = len(pair3_fill) if pair3_fill else 0

                def pop_fills():
                    done[0] += 1
                    target = total_fill * min(done[0], drain_steps) // drain_steps
                    while fill and total_fill - len(fill) < target:
                        fill.pop(0)()

                def pop_p3(step):
                    # drain the tail chains over steps 2..nkt of pair 3
                    if not pair3_fill:
                        return
                    span = nkt - 2
                    t = p3_total * min(max(step - 1, 0), span) // span
                    while pair3_fill and p3_total - len(pair3_fill) < t:
                        pair3_fill.pop(0)()

                nfill = []

                def norm(pair, last=False):
                    """Normalization stage-out. DVE time scales with the free
                    dim, not partitions, so each pos bank leaves PSUM in ONE
                    full-width copy (pos recycles after a single op — the
                    next pair's AVs start ~1.4us earlier than with split
                    copies) and the recips/muls read sub-ranges of the staged
                    tiles directly. Head A: stg0 = [y | d]; head B:
                    stg1 = [d | y]. The recips/muls are queued onto `nfill`
                    and paced one per kt-step of the NEXT pair, so they don't
                    monopolize the in-order DVE queue right when the next
                    pair's stage needs it. The final pair instead runs
                    immediately, using PE-swap matmuls (64-lane permutation)
                    in place of the slow SBUF->SBUF lane-shift DMAs."""
                    stg1 = smp.tile([128, CHUNK], F32, tag="nrm", bufs=12, name="stg1")
                    rcA = smp.tile([128, CHUNK], F32, tag="nrm", bufs=12, name="rcA")
                    rcB = smp.tile([128, CHUNK], F32, tag="nrm", bufs=12, name="rcB")
                    if last:
                        # bf16 staging: the PE swap runs at 1 cyc/row instead
                        # of fp32's 4 (215ns vs 853ns on the critical chain);
                        # bf16 rounding of one pair's denominators is ~0.4%
                        # on 1/16 of the last chunk — noise vs the 2e-2
                        # budget. The swap's PSUM output is f32 either way.
                        stg0 = smp.tile(
                            [128, CHUNK], BF16, tag="nrm", bufs=12, name="stg0b"
                        )
                        nc.vector.tensor_copy(stg0[:], pos[0][:])
                        nc.scalar.copy(stg1[:], pos[1][:])
                        # dA = swap(stg0)[0:64] on the PE; both pos banks are
                        # already free, so their pool slots host the swaps
                        swA = pop.tile([128, CHUNK], F32, tag="po", name="swA")
                        nc.tensor.matmul(
                            swA[:], swap_bf[:], stg0[:], start=True, stop=True
                        )
                        nc.vector.reciprocal_approx_fast(rcB[0:64, :], stg1[0:64, :])
                        swB = pop.tile([128, CHUNK], F32, tag="po", name="swB")
                        nc.tensor.matmul(
                            swB[64:128, :],
                            swap_sb[0:64, 64:128],
                            rcB[0:64, :],
                            start=True,
                            stop=True,
                        )
                        # reciprocal straight from the swapped PSUM — saves a
                        # 0.7us staging copy on the critical path
                        nc.vector.reciprocal_approx_fast(rcA[0:64, :], swA[0:64, :])
                        nc.vector.tensor_mul(
                            ytc[0:64, pair, :], stg0[0:64, :], rcA[0:64, :]
                        )
                        nc.vector.tensor_mul(
                            ytc[64:128, pair, :], stg1[64:128, :], swB[64:128, :]
                        )
                        return
                    stg0 = smp.tile([128, CHUNK], F32, tag="nrm", bufs=12, name="stg0")
                    dnA = smp.tile([128, CHUNK], F32, tag="nrm", bufs=12, name="dnA")
                    nc.vector.tensor_copy(stg0[:], pos[0][:])
                    nc.vector.tensor_copy(stg1[:], pos[1][:])
                    nfill.extend(
                        [
                            lambda: nc.gpsimd.dma_start(
                                dnA[0:64, :], stg0[64:128, :]
                            ),
                            lambda: nc.vector.reciprocal_approx_fast(
                                rcB[0:64, :], stg1[0:64, :]
                            ),
                            lambda: nc.gpsimd.dma_start(
                                rcB[64:128, :], rcB[0:64, :]
                            ),
                            lambda: nc.vector.reciprocal_approx_fast(
                                rcA[0:64, :], dnA[0:64, :]
                            ),
                            lambda: nc.vector.tensor_mul(
                                ytc[0:64, pair, :], stg0[0:64, :], rcA[0:64, :]
                            ),
                            lambda: nc.vector.tensor_mul(
                                ytc[64:128, pair, :], stg1[64:128, :], rcB[64:128, :]
                            ),
                        ]
                    )

                # pre-pop fills ahead of the first score matmul (it waits on
                # the QKV cp chain; without this it head-of-line blocks
                # ready fill work in the in-order PE queue)
                for _ in range(5):
                    if fill:
                        fill.pop(0)()
                for pair in range(4):  # heads (2*pair, 2*pair+1)
                    pos = [
                        pop.tile([128, CHUNK], F32, tag="po", name=f"po{s}")
                        for s in range(2)
                    ]
                    pending = None
                    for kt in range(nkt):
                        kc, lk = kt // 4, kt % 4
                        j = kt - 4 * qc
                        w = CHUNK if j < 0 else CHUNK - j * 128
                        q0 = CHUNK - w
                        sps2 = spsp.tile([128, 2 * CHUNK], F32, tag="sps")
                        for s in range(2):  # the two heads of the pair
                            off = s * 64
                            nc.tensor.matmul(
                                sps2[:, s * CHUNK : s * CHUNK + w],
                                KTc[kc][off : off + 64, pair, lk * 128 : (lk + 1) * 128],
                                QTc[off : off + 64, pair, q0:CHUNK],
                                start=True,
                                stop=True,
                                tile_position=(off, 0),
                            )
                        pt = ptp.tile([128, 2 * CHUNK], BF16)
                        nc.scalar.activation(
                            pt[:, 0 : CHUNK + w],
                            sps2[:, 0 : CHUNK + w],
                            EXP,
                            scale=SCALE,
                        )
                        if j >= 0:
                            for s in range(2):
                                nc.gpsimd.affine_select(
                                    out=pt[:, s * CHUNK : s * CHUNK + w],
                                    in_=pt[:, s * CHUNK : s * CHUNK + w],
                                    compare_op=mybir.AluOpType.is_ge,
                                    fill=0.0,
                                    base=0,
                                    pattern=[[1, w]],
                                    channel_multiplier=-1,
                                )
                        pop_fills()
                        if nfill:
                            nfill.pop(0)()
                        if pair == 3:
                            pop_p3(kt)
                        if pending is not None:
                            pending()

                        def make_av(kt=kt, kc=kc, lk=lk, pt=pt, w=w, q0=q0):
                            for s in range(2):
                                h = 2 * pair + s
                                nc.tensor.matmul(
                                    pos[s][:, q0:CHUNK],
                                    VAc[kc][:, lk, h],
                                    pt[:, s * CHUNK : s * CHUNK + w],
                                    start=(kt == 0),
                                    stop=(kt == nkt - 1),
                                )

                        pending = make_av
                    pending()
                    norm(pair, last=(last_chunk and pair == 3))
                    # pre-pop a few fills so the next pair's first score
                    # matmul (which waits on this pair's normalization /
                    # exp pipeline) doesn't head-of-line block independent
                    # work in the in-order PE queue
                    for _ in range(3):
                        if fill:
                            fill.pop(0)()
                while fill:
                    fill.pop(0)()
                while pair3_fill:
                    pair3_fill.pop(0)()
                while nfill:
                    nfill.pop(0)()

            # prologue: x(0) first so QKV(0) starts ASAP; wo last (needed
            # only from the ch=2 window on)
            load_x(0)  # x(0) rides the sync queue alone
            _dma_engs = [nc.gpsimd, nc.scalar, nc.sync]
            _dq = [0]

            def _dma(dst, srcslice, n=3):
                _dma_engs[_dq[0] % n].dma_start(dst, srcslice)
                _dq[0] += 1

            # pair-0-critical first, on gpsimd+scalar only (parallel to x(0))
            for kt in range(KT_C):
                _dma(wqq[kt][0][:], wq_re[:, kt, 0:128], n=2)
                _dma(wqk[kt][0][:], wq_re[:, kt, GF : GF + 128], n=2)
            for kt in range(KT_C):
                _dma(wqv[kt][:], wq_re[:, kt, 2 * GF : 3 * GF], n=2)
            for g in (1, 2, 3):
                for kt in range(KT_C):
                    _dma(wqq[kt][g][:], wq_re[:, kt, g * 128 : (g + 1) * 128])
                    _dma(wqk[kt][g][:], wq_re[:, kt, GF + g * 128 : GF + (g + 1) * 128])
            for kt in range(GF // 128):
                _dma(wo_sb[:, kt], wo_re[:, kt])
            _dma(swap_sb[:], swapC.ap()[:, :])
            nc.vector.tensor_copy(swap_bf[:], swap_sb[:])
            load_x(1)  # prefetch behind the weights on the rotated queues
            for t in qkv_thunks(0, spread=True):
                t()
            for ch in range(NCHUNK):
                if ch + 2 < NCHUNK:
                    load_x(ch + 2)
                fill = []
                if ch + 1 < NCHUNK:
                    fill += qkv_thunks(ch + 1)
                p3fill = None
                if ch == NCHUNK - 1:
                    # all three finished chunks' WO projections fill the
                    # last attention phase (which would otherwise run too
                    # close to exp-bound and let the HAM clock gate drop).
                    # Chains interleave PAIRWISE so at most two are ever
                    # open on the two proj PSUM slots — a third in flight
                    # would head-of-line block the in-order PE queue.
                    chains = [c for tri in zip(*(wo_chains(i) for i in range(3)))
                              for c in tri]
                    for i in range(0, len(chains), 2):
                        fill += [u for p in zip(chains[i], chains[i + 1]) for u in p]
                    early3, late3 = wo_tail_units()
                    p3fill = early3
                attention(ch, fill, p3fill)
            for t in late3:
                t()

    nc.compile()
    return nc


def _prep_inputs(x, wqkv, wo):
    """Per-core input maps: core c = (batch c // 2, head-group c % 2)."""
    import ml_dtypes

    bf16 = ml_dtypes.bfloat16
    x = np.asarray(x, dtype=np.float32).astype(bf16)
    wqkv = np.asarray(wqkv, dtype=np.float32).astype(bf16)
    wo = np.asarray(wo, dtype=np.float32).astype(bf16)
    # 64-lane swap permutation: (swapC.T @ v)[p] = v[(p + 64) % 128]
    swap_mat = np.zeros((128, 128), dtype=np.float32)
    swap_mat[(np.arange(128) + 64) % 128, np.arange(128)] = 1
    in_maps = []
    for c in range(8):
        b, g = c // 2, c % 2
        rows = np.r_[
            g * GF : (g + 1) * GF,
            C + g * GF : C + (g + 1) * GF,
            2 * C + g * GF : 2 * C + (g + 1) * GF,
        ]
        in_maps.append(
            {
                "xT": np.ascontiguousarray(x[b].T),
                "wqkvT": np.ascontiguousarray(wqkv[rows].T),
                "woT": np.ascontiguousarray(wo[:, g * GF : (g + 1) * GF].T),
                "swapC": swap_mat,
            }
        )
    return in_maps


def _run(x, wqkv, wo, trace=False, trace_cores=None):
    if "nc" not in _CACHE:
        _CACHE["nc"] = _build()
    res = run_bass_kernel_spmd(
        _CACHE["nc"],
        _prep_inputs(x, wqkv, wo),
        core_ids=list(range(8)),
        trace=trace,
        trace_cores=trace_cores,
    )
    out = np.empty((B, T, C), dtype=np.float32)
    for b in range(B):
        r0, r1 = res.results[2 * b], res.results[2 * b + 1]
        o = r0["outT"].astype(np.float32) + r1["outT"].astype(np.float32)
        o[:, 3 * CHUNK :] += r0["outT2"].astype(np.float32) + r1["outT2"].astype(
            np.float32
        )
        out[b] = o.T
    return out, res


def kernel(x, wqkv, wo):
    out, _ = _run(x, wqkv, wo)
    return out



# revision 62
# speedup vs baseline: 1.1896x; 1.1896x over previous
"""Causal multi-head attention (B=4, T=2048, C=1024, H=16) on 8 TRN2 cores.

Sharding: core c = (batch b = c // 2, head-group g = c % 2). Each core runs
QKV projection + attention + its half of the output projection for one
batch and 8 heads (Megatron-style column-parallel wqkv / row-parallel wo);
the two partial wo products per batch are summed on the host during
unsharding.

Device layouts (per core):
  xT    [C, T]        x[b] transposed (C on partitions for QKV matmuls)
  wqkvT [C, 3*512]    head-group slice of wqkv, transposed; cols [Q|K|V]
  woT   [512, C]      wo columns for this head-group, transposed
  outT  [C, T]        partial (x @ wqkv.T -> attn -> @ wo.T).T

All matmuls run in float32r (tf32-like: ~1.5e-4 rel err, ~3.3x faster
than fp32 on the PE). Scores are computed transposed (k on partitions) so
softmax P.T feeds the AV matmul directly with no transposes; the softmax
denominator rides along as a 65th row of V (column of ones), and the
causal mask is an affine_select on the diagonal score tiles. exp() is
applied without max-subtraction: scores are ~N(0,1), far inside fp32 exp
range.

Scheduling: the T dimension is processed in four 512-chunks. Attention on
chunk ch is emitted with the (PE-only) QKV projection of chunk ch+1 and
the WO projection of chunk ch-1 round-robin-interleaved into its
(head-pair, k-tile) loop, so the PE array keeps streaming while ACT runs
the softmax exps — that keeps the HAM clock gate at 2.4 GHz. Heads are
processed in pairs living at partition offsets 0/64: their K=64 score
matmuls target disjoint row-halves of the PE array and run concurrently.

Edges: a few dependency-free warm-up matmuls run during the input-DMA
fill so the HAM clock gate is already at 2.4 GHz when the first QKV
chain's data lands. Each pair's normalization leaves PSUM in one
full-width copy per pos bank (pos recycles after a single DVE op, so the
next pair's AV matmuls aren't gated ~2.7us behind four split copies) and
its recips/muls are paced one-per-step into the next pair's loop so they
don't monopolize the in-order DVE queue. A few fill units are pre-popped
ahead of each pair's first score matmul, which otherwise head-of-line
blocks ready work in the in-order PE queue. All WO projections fill the
last attention phase. The tail: the last chunk's WO chains accumulate
kt0-1 during the final pair's loop (partial straight to outT), kt2
streams during the final normalization (keeping the clock gate warm),
and only kt3 waits for it, stored to outT2 for the host to add back.
That final normalization swaps denominator lanes with PE permutation
matmuls (swapC) instead of ~1.7us SBUF->SBUF DMAs. outT is bf16 (host
sums partials in f32), halving store traffic; stores alternate between
the sync and scalar DMA queues (never gpsimd, which carries the
latency-critical normalization lane shifts).
"""

import sys

sys.path.insert(0, "/opt/trn_rl_repo")

import numpy as np

import concourse.bass as bass  # noqa: F401  (AP helpers)
import concourse.bass_utils as _bu
import concourse.mybir as mybir
import concourse.tile as tile
from concourse import bacc
from concourse.bass_utils import run_bass_kernel_spmd

# NOTE: walrus's --enable-ldw-opt=true (used by an earlier fp32r version of
# this kernel) rejects bf16 LDWEIGHTS ("InstLdweights is not compatible with
# LDW optimization"), so this kernel runs with the concourse default (off).

F32 = mybir.dt.float32
F32R = mybir.dt.float32r
BF16 = mybir.dt.bfloat16
EXP = mybir.ActivationFunctionType.Exp

B, T, C, H = 4, 2048, 1024, 16
HD = 64  # head dim
HPC = 8  # heads per core
GF = HPC * HD  # 512 group features
CHUNK = 512
NCHUNK = T // CHUNK  # 4
KT_C = C // 128  # 8 k-tiles over C
SCALE = 1.0 / np.sqrt(HD)

_CACHE = {}


def _build():
    nc = bacc.Bacc(
        "TRN2", target_bir_lowering=False, debug=False, num_devices=8
    )
    xT = nc.dram_tensor("xT", [C, T], BF16, kind="ExternalInput")
    wqkvT = nc.dram_tensor("wqkvT", [C, 3 * GF], BF16, kind="ExternalInput")
    woT = nc.dram_tensor("woT", [GF, C], BF16, kind="ExternalInput")
    # 64-lane swap permutation (host-built): swapC.T @ x swaps the two
    # partition halves of x on the PE — used by the final normalization
    # instead of a ~1.7us SBUF->SBUF lane-shift DMA
    swapC = nc.dram_tensor("swapC", [128, 128], F32, kind="ExternalInput")
    # bf16 output halves the store traffic (the two per-batch partials are
    # summed on the host in f32; the extra rounding is ~3e-3 rel, well
    # inside the 2e-2 budget)
    outT = nc.dram_tensor("outT", [C, T], BF16, kind="ExternalOutput")
    # kt3 (last head-pair) contribution of the last chunk's WO, stored
    # separately so the chunk-3 WO partials can stream out before the final
    # normalization; host adds it onto outT[:, 3*CHUNK:].
    outT2 = nc.dram_tensor("outT2", [C, CHUNK], BF16, kind="ExternalOutput")

    xT_re = xT.ap().rearrange("(kt p) t -> p kt t", p=128)
    wq_re = wqkvT.ap().rearrange("(kt p) f -> p kt f", p=128)
    wo_re = woT.ap().rearrange("(kt p) f -> p kt f", p=128)
    outT2_re = outT2.ap().rearrange("(g two p) t -> p g two t", p=128, two=2)

    with tile.TileContext(nc) as tc:
        with (
            tc.tile_pool(name="weights", bufs=1) as wp,
            tc.tile_pool(name="persist", bufs=1) as persist,
            tc.tile_pool(name="xin", bufs=2) as xp,
            tc.tile_pool(name="qt", bufs=2) as qtp,
            tc.tile_pool(name="pt", bufs=2) as ptp,
            tc.tile_pool(name="yt", bufs=4) as ytp,
            tc.tile_pool(name="small", bufs=2) as smp,
            tc.tile_pool(name="ost", bufs=4) as ostp,
            tc.tile_pool(name="ost2", bufs=4) as ost2p,
            tc.tile_pool(name="proj", bufs=2, space="PSUM") as projp,
            tc.tile_pool(name="sps", bufs=2, space="PSUM") as spsp,
            tc.tile_pool(name="pops", bufs=2, space="PSUM") as pop,
        ):
            # wq split column-wise: [Q four 128-col tiles][K four][V one 512-col]
            wqq = [[wp.tile([128, 128], BF16, name=f"wqq{kt}_{g}") for g in range(4)]
                   for kt in range(KT_C)]
            wqk = [[wp.tile([128, 128], BF16, name=f"wqk{kt}_{g}") for g in range(4)]
                   for kt in range(KT_C)]
            wqv = [wp.tile([128, GF], BF16, name=f"wqv{kt}") for kt in range(KT_C)]
            wo_sb = wp.tile([128, GF // 128, C], BF16)

            # per-chunk persistent K.T / V_aug. V is padded to 128 columns
            # with a 64-wide all-ones block: the AV matmul then emits the
            # softmax denominator replicated across 64 PSUM partitions, so
            # normalization needs no gpsimd partition_broadcast (no gpsimd
            # library thrash). Even heads are [V | ones] (y on lanes 0-63),
            # odd heads [ones | V] (y on lanes 64-127, where ytc wants it) —
            # DVE ops must be lane-aligned on HW, and the custom reciprocal
            # only works at partition base 0, so each head needs exactly one
            # DMA lane-shift of 1/d (resp. d) across the 64-lane boundary.
            KTc = [persist.tile([128, 4, CHUNK], BF16, name=f"KT{i}") for i in range(NCHUNK)]
            VAc = [persist.tile([128, 4, HPC, 2 * HD], BF16, name=f"VA{i}") for i in range(NCHUNK)]

            # HAM pre-warm: a few dependency-free matmuls on a memset tile
            # keep the PE busy through the input-DMA fill, so the 4096-cycle
            # activity window flips the clock gate to 2.4 GHz right as the
            # first QKV chain's data lands (otherwise the first ~3.4us of
            # real matmuls run at 1.2 GHz). Wide 512-col dummies: enough
            # cold-clock runtime to warm the gate without clogging the
            # in-order PE queue past the point where real data arrives.
            warm = wp.tile([128, CHUNK], BF16, name="warm")
            swap_sb = wp.tile([128, 128], F32, name="swap")
            swap_bf = wp.tile([128, 128], BF16, name="swapbf")
            nc.vector.memset(warm[:], 0.25)
            warm_ps = spsp.tile([128, 2 * CHUNK], F32, tag="sps", name="warmps")
            for _ in range(7):
                nc.tensor.matmul(
                    warm_ps[:, 0:CHUNK], warm[:, 0:128], warm[:], start=True, stop=True
                )

            for i in range(NCHUNK):
                for h in range(HPC):
                    lo = HD if h % 2 == 0 else 0
                    nc.vector.memset(VAc[i][:, :, h, lo : lo + HD], 1.0)

            xts = {}
            QTcs = {}
            ytcs = {}

            def load_x(ch):
                xts[ch] = [
                    xp.tile([128, CHUNK], BF16, tag="xt", bufs=24, name=f"xt{ch}_{kt}")
                    for kt in range(KT_C)
                ]
                for kt in range(KT_C):
                    nc.sync.dma_start(
                        xts[ch][kt][:],
                        xT_re[:, kt, ch * CHUNK : (ch + 1) * CHUNK],
                    )

            def qkv_thunks(ch, spread=False):
                """Projection work for chunk ch as ~108 single-instruction
                units (8 matmuls + 1 copy per chain, 12 chains) so fills can
                slot between dependent attention steps at matmul granularity."""
                QTcs[ch] = qtp.tile([128, 4, CHUNK], BF16, tag="qtc", name=f"qtc{ch}")
                units = []
                seq = [0]

                def chain_psum(st):
                    if not spread:
                        return projp.tile([128, CHUNK], F32, tag="proj", name="ps")
                    i = seq[0]
                    seq[0] += 1
                    # spread over projp+pop only: the spsp slots must only
                    # ever hold uniform [128, 2*CHUNK] sps2 tiles, else the
                    # pool's extent-based reuse deps miss orderings
                    pool, tag = [
                        (projp, "proj"), (projp, "proj"), (pop, "po"), (pop, "po"),
                    ][i % 4]
                    return pool.tile([128, CHUNK], F32, tag=tag, name=f"pps{ch}_{i}")

                def qk_units(m):
                    st = {}

                    def mm(kt, m=m, st=st):
                        if kt == 0:
                            st["ps"] = chain_psum(st)
                        wt = wqq[kt][m] if m < 4 else wqk[kt][m - 4]
                        nc.tensor.matmul(
                            st["ps"][:],
                            wt[:],
                            xts[ch][kt][:],
                            start=(kt == 0),
                            stop=(kt == KT_C - 1),
                        )

                    def cp(m=m, st=st):
                        dst = QTcs[ch][:, m] if m < 4 else KTc[ch][:, m - 4]
                        nc.vector.tensor_copy(dst, st["ps"][:])

                    return [lambda kt=kt: mm(kt) for kt in range(KT_C)] + [cp]

                def v_units(t4):
                    st = {}

                    def mm(kt, t4=t4, st=st):
                        if kt == 0:
                            st["ps"] = chain_psum(st)
                        nc.tensor.matmul(
                            st["ps"][:],
                            xts[ch][kt][:, t4 * 128 : (t4 + 1) * 128],
                            wqv[kt][:],
                            start=(kt == 0),
                            stop=(kt == KT_C - 1),
                        )

                    def cp(t4=t4, st=st):
                        src = st["ps"][:].rearrange(
                            "p (h4 two d) -> p two h4 d", two=2, d=HD
                        )
                        dst = VAc[ch][:, t4].rearrange(
                            "p (h4 two) c -> p two h4 c", two=2
                        )
                        nc.vector.tensor_copy(dst[:, 0, :, 0:HD], src[:, 0])
                        nc.vector.tensor_copy(dst[:, 1, :, HD : 2 * HD], src[:, 1])

                    return [lambda kt=kt: mm(kt) for kt in range(KT_C)] + [cp]

                if spread:
                    # prologue (chunk 0): kt-major pairwise emission for the
                    # leading chains, matching the x-tile DMA arrival order —
                    # chain-major would park the in-order PE queue on chain
                    # 0's last tiles while later chains' early tiles (already
                    # in SBUF) sit ready behind it
                    uq0, uk0 = qk_units(0), qk_units(4)
                    uv = [v_units(t4) for t4 in range(4)]
                    units += [u for p in zip(uq0, uk0) for u in p]
                    units += [u for p in zip(uv[0], uv[1]) for u in p]
                    units += [u for p in zip(uv[2], uv[3]) for u in p]
                else:
                    units += qk_units(0)
                    units += qk_units(4)
                    for t4 in range(4):
                        units += v_units(t4)
                for m in (1, 5, 2, 6, 3, 7):
                    units += qk_units(m)
                return units

            # store DMAs alternate between the sync and scalar queues so
            # back-to-back stores don't serialize on one engine's ~600ns
            # descriptor-gen time. The gpsimd queue is deliberately excluded:
            # it carries the normalization lane-shift DMAs, whose latency is
            # on the critical path and must not sit behind 128KB store
            # transfers.
            _st_engs = [nc.sync, nc.scalar]
            _stq = [0]

            def _store(dst, src):
                _st_engs[_stq[0] % 2].dma_start(dst, src)
                _stq[0] += 1

            def wo_chains(ch):
                """Output projection for chunk ch as 8 chains of 5
                single-instruction units (4 matmuls + 1 copy/dma)."""
                cs = slice(ch * CHUNK, (ch + 1) * CHUNK)

                def wo_units(m):
                    st = {}

                    def mm(kt, m=m, st=st):
                        if kt == 0:
                            st["ps"] = projp.tile(
                                [128, CHUNK], F32, tag="proj", name=f"wop{ch}_{m}"
                            )
                        nc.tensor.matmul(
                            st["ps"][:],
                            wo_sb[:, kt, m * 128 : (m + 1) * 128],
                            ytcs[ch][:, kt],
                            start=(kt == 0),
                            stop=(kt == GF // 128 - 1),
                        )

                    def out(m=m, st=st):
                        ot = ostp.tile([128, CHUNK], BF16)
                        nc.vector.tensor_copy(ot[:], st["ps"][:])
                        _store(outT[m * 128 : (m + 1) * 128, cs], ot[:])

                    return [lambda kt=kt: mm(kt) for kt in range(GF // 128)] + [out]

                return [wo_units(m) for m in range(8)]

            def wo_tail_units():
                """Last chunk's WO, split so the final normalization gates
                little PE work: `early` (drained during the last head-pair's
                kt loop; depends only on pairs 0-1, normalized long before)
                accumulates kt0-1 of every chain and stores that partial
                straight to outT; `late` (after pair 2/3 normalize) runs the
                kt2+kt3 matmuls, stored to outT2 for the host to add back."""
                ch = NCHUNK - 1
                cs = slice(ch * CHUNK, (ch + 1) * CHUNK)
                early, late = [], []
                for m in range(8):
                    st = {}

                    def mm01(kt, m=m, st=st):
                        if kt == 0:
                            st["ps"] = projp.tile(
                                [128, CHUNK], F32, tag="proj", name=f"wop3e_{m}"
                            )
                        nc.tensor.matmul(
                            st["ps"][:],
                            wo_sb[:, kt, m * 128 : (m + 1) * 128],
                            ytcs[ch][:, kt],
                            start=(kt == 0),
                            stop=(kt == 1),
                        )

                    def outp(m=m, st=st):
                        ot = ostp.tile([128, CHUNK], BF16)
                        # alternate DVE/ACT: during the last pair's loop the
                        # DVE is also running the previous pair's norm ops,
                        # and a copy stuck behind them would stall the
                        # 2-slot psum wave rotation (and with it the PE)
                        cp = nc.vector.tensor_copy if m % 2 == 0 else nc.scalar.copy
                        cp(ot[:], st["ps"][:])
                        _store(outT[m * 128 : (m + 1) * 128, cs], ot[:])

                    early += [lambda kt=kt, mm01=mm01: mm01(kt) for kt in range(2)]
                    early.append(outp)
                # Every late chain gets its own dedicated PSUM region — the
                # spsp banks (free once the last score/exp retires), the pos
                # banks (free once the stage-out copies run) and the projp
                # slots. That way all eight kt2 matmuls (which only need pair
                # 2) stream during the final normalization, keeping the HAM
                # clock gate warm, and only the kt3s wait for it.
                shared = {}

                def region(c, shared=shared):
                    if c < 4:
                        i = c // 2
                        if f"sps{i}" not in shared:
                            shared[f"sps{i}"] = spsp.tile(
                                [128, 2 * CHUNK], F32, tag="sps", name=f"wot{i}"
                            )
                        return shared[f"sps{i}"][:, (c % 2) * CHUNK : (c % 2 + 1) * CHUNK]
                    if c < 6:
                        k = f"po{c}"
                        if k not in shared:
                            shared[k] = pop.tile([128, CHUNK], F32, tag="po", name=k)
                        return shared[k][:]
                    k = f"pj{c}"
                    if k not in shared:
                        shared[k] = projp.tile([128, CHUNK], F32, tag="proj", name=k)
                    return shared[k][:]

                def mk_mm(m, kt):
                    def mm():
                        nc.tensor.matmul(
                            region(m),
                            wo_sb[:, kt, m * 128 : (m + 1) * 128],
                            ytcs[ch][:, kt],
                            start=(kt == 2),
                            stop=(kt == 3),
                        )

                    return mm

                def mk_out(m):
                    # chains pair up into one [128, 2*CHUNK] staging tile and
                    # one store, halving the tail's DMA issue serialization;
                    # the two copies ride different engines
                    def out3():
                        if f"ot{m // 2}" not in shared:
                            shared[f"ot{m // 2}"] = ost2p.tile(
                                [128, 2 * CHUNK], BF16, name=f"ot{m // 2}"
                            )
                        ot = shared[f"ot{m // 2}"]
                        cp = nc.scalar.copy if m % 2 == 0 else nc.vector.tensor_copy
                        cp(ot[:, (m % 2) * CHUNK : (m % 2 + 1) * CHUNK], region(m))
                        if m % 2 == 1:
                            _store(
                                outT2_re[:, m // 2], ot[:].rearrange(
                                    "p (two t) -> p two t", two=2
                                )
                            )

                    return out3

                late += [mk_mm(m, 2) for m in range(8)]
                for m in range(8):
                    late += [mk_mm(m, 3), mk_out(m)]
                return early, late

            def attention(qc, fill, pair3_fill=None):
                """Attention for q-chunk qc. The AV matmul for k-tile kt is
                emitted one step late (after the kt+1 score matmuls and any
                fill units), so the PE never sits behind ACT's exp in the
                in-order engine queue; `fill` units drain proportionally.
                `pair3_fill` units (which depend on pairs 0-2 being done)
                drain during the last pair's k-tile loop."""
                ytcs[qc] = ytp.tile([128, GF // 128, CHUNK], BF16, tag="ytc", name=f"ytc{qc}")
                ytc = ytcs[qc]
                QTc = QTcs[qc]
                last_chunk = qc == NCHUNK - 1
                nkt = (qc + 1) * 4
                nsteps = 4 * nkt
                total_fill = len(fill)
                done = [0]

                # for the last chunk, stretch the w-fill drain 2 steps into
                # pair 3 so the tail-chain drain takes over seamlessly
                drain_steps = (3 * nkt + 2) if last_chunk else nsteps
                p3_total = len(pair3_fill) if pair3_fill else 0

                def pop_fills():
                    done[0] += 1
                    target = total_fill * min(done[0], drain_steps) // drain_steps
                    while fill and total_fill - len(fill) < target:
                        fill.pop(0)()

                def pop_p3(step):
                    # drain the tail chains over steps 2..nkt of pair 3
                    if not pair3_fill:
                        return
                    span = nkt - 2
                    t = p3_total * min(max(step - 1, 0), span) // span
                    while pair3_fill and p3_total - len(pair3_fill) < t:
                        pair3_fill.pop(0)()

                nfill = []

                def norm(pair, last=False):
                    """Normalization stage-out. DVE time scales with the free
                    dim, not partitions, so each pos bank leaves PSUM in ONE
                    full-width copy (pos recycles after a single op — the
                    next pair's AVs start ~1.4us earlier than with split
                    copies) and the recips/muls read sub-ranges of the staged
                    tiles directly. Head A: stg0 = [y | d]; head B:
                    stg1 = [d | y]. The recips/muls are queued onto `nfill`
                    and paced one per kt-step of the NEXT pair, so they don't
                    monopolize the in-order DVE queue right when the next
                    pair's stage needs it. The final pair instead runs
                    immediately, using PE-swap matmuls (64-lane permutation)
                    in place of the slow SBUF->SBUF lane-shift DMAs."""
                    stg1 = smp.tile([128, CHUNK], F32, tag="nrm", bufs=12, name="stg1")
                    rcA = smp.tile([128, CHUNK], F32, tag="nrm", bufs=12, name="rcA")
                    rcB = smp.tile([128, CHUNK], F32, tag="nrm", bufs=12, name="rcB")
                    if last:
                        # bf16 staging: the PE swap runs at 1 cyc/row instead
                        # of fp32's 4 (215ns vs 853ns on the critical chain);
                        # bf16 rounding of one pair's denominators is ~0.4%
                        # on 1/16 of the last chunk — noise vs the 2e-2
                        # budget. The swap's PSUM output is f32 either way.
                        stg0 = smp.tile(
                            [128, CHUNK], BF16, tag="nrm", bufs=12, name="stg0b"
                        )
                        nc.vector.tensor_copy(stg0[:], pos[0][:])
                        nc.scalar.copy(stg1[:], pos[1][:])
                        # dA = swap(stg0)[0:64] on the PE; both pos banks are
                        # already free, so their pool slots host the swaps
                        swA = pop.tile([128, CHUNK], F32, tag="po", name="swA")
                        nc.tensor.matmul(
                            swA[:], swap_bf[:], stg0[:], start=True, stop=True
                        )
                        nc.vector.reciprocal_approx_fast(rcB[0:64, :], stg1[0:64, :])
                        swB = pop.tile([128, CHUNK], F32, tag="po", name="swB")
                        nc.tensor.matmul(
                            swB[64:128, :],
                            swap_sb[0:64, 64:128],
                            rcB[0:64, :],
                            start=True,
                            stop=True,
                        )
                        # reciprocal straight from the swapped PSUM — saves a
                        # 0.7us staging copy on the critical path
                        nc.vector.reciprocal_approx_fast(rcA[0:64, :], swA[0:64, :])
                        nc.vector.tensor_mul(
                            ytc[0:64, pair, :], stg0[0:64, :], rcA[0:64, :]
                        )
                        nc.vector.tensor_mul(
                            ytc[64:128, pair, :], stg1[64:128, :], swB[64:128, :]
                        )
                        return
                    stg0 = smp.tile([128, CHUNK], F32, tag="nrm", bufs=12, name="stg0")
                    dnA = smp.tile([128, CHUNK], F32, tag="nrm", bufs=12, name="dnA")
                    nc.vector.tensor_copy(stg0[:], pos[0][:])
                    nc.vector.tensor_copy(stg1[:], pos[1][:])
                    nfill.extend(
                        [
                            lambda: nc.gpsimd.dma_start(
                                dnA[0:64, :], stg0[64:128, :]
                            ),
                            lambda: nc.vector.reciprocal_approx_fast(
                                rcB[0:64, :], stg1[0:64, :]
                            ),
                            lambda: nc.gpsimd.dma_start(
                                rcB[64:128, :], rcB[0:64, :]
                            ),
                            lambda: nc.vector.reciprocal_approx_fast(
                                rcA[0:64, :], dnA[0:64, :]
                            ),
                            lambda: nc.vector.tensor_mul(
                                ytc[0:64, pair, :], stg0[0:64, :], rcA[0:64, :]
                            ),
                            lambda: nc.vector.tensor_mul(
                                ytc[64:128, pair, :], stg1[64:128, :], rcB[64:128, :]
                            ),
                        ]
                    )

                # pre-pop fills ahead of the first score matmul (it waits on
                # the QKV cp chain; without this it head-of-line blocks
                # ready fill work in the in-order PE queue)
                for _ in range(5):
                    if fill:
                        fill.pop(0)()
                for pair in range(4):  # heads (2*pair, 2*pair+1)
                    pos = [
                        pop.tile([128, CHUNK], F32, tag="po", name=f"po{s}")
                        for s in range(2)
                    ]
                    pending = None
                    for kt in range(nkt):
                        kc, lk = kt // 4, kt % 4
                        j = kt - 4 * qc
                        w = CHUNK if j < 0 else CHUNK - j * 128
                        q0 = CHUNK - w
                        sps2 = spsp.tile([128, 2 * CHUNK], F32, tag="sps")
                        for s in range(2):  # the two heads of the pair
                            off = s * 64
                            nc.tensor.matmul(
                                sps2[:, s * CHUNK : s * CHUNK + w],
                                KTc[kc][off : off + 64, pair, lk * 128 : (lk + 1) * 128],
                                QTc[off : off + 64, pair, q0:CHUNK],
                                start=True,
                                stop=True,
                                tile_position=(off, 0),
                            )
                        pt = ptp.tile([128, 2 * CHUNK], BF16)
                        nc.scalar.activation(
                            pt[:, 0 : CHUNK + w],
                            sps2[:, 0 : CHUNK + w],
                            EXP,
                            scale=SCALE,
                        )
                        if j >= 0:
                            for s in range(2):
                                nc.gpsimd.affine_select(
                                    out=pt[:, s * CHUNK : s * CHUNK + w],
                                    in_=pt[:, s * CHUNK : s * CHUNK + w],
                                    compare_op=mybir.AluOpType.is_ge,
                                    fill=0.0,
                                    base=0,
                                    pattern=[[1, w]],
                                    channel_multiplier=-1,
                                )
                        pop_fills()
                        if nfill:
                            nfill.pop(0)()
                        if pair == 3:
                            pop_p3(kt)
                        if pending is not None:
                            pending()

                        def make_av(kt=kt, kc=kc, lk=lk, pt=pt, w=w, q0=q0):
                            for s in range(2):
                                h = 2 * pair + s
                                nc.tensor.matmul(
                                    pos[s][:, q0:CHUNK],
                                    VAc[kc][:, lk, h],
                                    pt[:, s * CHUNK : s * CHUNK + w],
                                    start=(kt == 0),
                                    stop=(kt == nkt - 1),
                                )

                        pending = make_av
                    pending()
                    norm(pair, last=(last_chunk and pair == 3))
                    # pre-pop a few fills so the next pair's first score
                    # matmul (which waits on this pair's normalization /
                    # exp pipeline) doesn't head-of-line block independent
                    # work in the in-order PE queue
                    for _ in range(3):
                        if fill:
                            fill.pop(0)()
                while fill:
                    fill.pop(0)()
                while pair3_fill:
                    pair3_fill.pop(0)()
                while nfill:
                    nfill.pop(0)()

            # prologue: x(0) first so QKV(0) starts ASAP; wo last (needed
            # only from the ch=2 window on)
            load_x(0)  # x(0) rides the sync queue alone
            _dma_engs = [nc.gpsimd, nc.scalar, nc.sync]
            _dq = [0]

            def _dma(dst, srcslice, n=3):
                _dma_engs[_dq[0] % n].dma_start(dst, srcslice)
                _dq[0] += 1

            # pair-0-critical first, on gpsimd+scalar only (parallel to x(0))
            for kt in range(KT_C):
                _dma(wqq[kt][0][:], wq_re[:, kt, 0:128], n=2)
                _dma(wqk[kt][0][:], wq_re[:, kt, GF : GF + 128], n=2)
            for kt in range(KT_C):
                _dma(wqv[kt][:], wq_re[:, kt, 2 * GF : 3 * GF], n=2)
            for g in (1, 2, 3):
                for kt in range(KT_C):
                    _dma(wqq[kt][g][:], wq_re[:, kt, g * 128 : (g + 1) * 128])
                    _dma(wqk[kt][g][:], wq_re[:, kt, GF + g * 128 : GF + (g + 1) * 128])
            for kt in range(GF // 128):
                _dma(wo_sb[:, kt], wo_re[:, kt])
            _dma(swap_sb[:], swapC.ap()[:, :])
            nc.vector.tensor_copy(swap_bf[:], swap_sb[:])
            load_x(1)  # prefetch behind the weights on the rotated queues
            for t in qkv_thunks(0, spread=True):
                t()
            for ch in range(NCHUNK):
                if ch + 2 < NCHUNK:
                    load_x(ch + 2)
                fill = []
                if ch + 1 < NCHUNK:
                    fill += qkv_thunks(ch + 1)
                p3fill = None
                if ch == NCHUNK - 1:
                    # all three finished chunks' WO projections fill the
                    # last attention phase (which would otherwise run too
                    # close to exp-bound and let the HAM clock gate drop).
                    # Chains interleave PAIRWISE so at most two are ever
                    # open on the two proj PSUM slots — a third in flight
                    # would head-of-line block the in-order PE queue.
                    chains = [c for tri in zip(*(wo_chains(i) for i in range(3)))
                              for c in tri]
                    for i in range(0, len(chains), 2):
                        fill += [u for p in zip(chains[i], chains[i + 1]) for u in p]
                    early3, late3 = wo_tail_units()
                    p3fill = early3
                attention(ch, fill, p3fill)
            for t in late3:
                t()

    nc.compile()
    return nc


def _prep_inputs(x, wqkv, wo):
    """Per-core input maps: core c = (batch c // 2, head-group c % 2)."""
    import ml_dtypes

    bf16 = ml_dtypes.bfloat16
    x = np.asarray(x, dtype=np.float32).astype(bf16)
    wqkv = np.asarray(wqkv, dtype=np.float32).astype(bf16)
    wo = np.asarray(wo, dtype=np.float32).astype(bf16)
    # 64-lane swap permutation: (swapC.T @ v)[p] = v[(p + 64) % 128]
    swap_mat = np.zeros((128, 128), dtype=np.float32)
    swap_mat[(np.arange(128) + 64) % 128, np.arange(128)] = 1
    in_maps = []
    for c in range(8):
        b, g = c // 2, c % 2
        rows = np.r_[
            g * GF : (g + 1) * GF,
            C + g * GF : C + (g + 1) * GF,
            2 * C + g * GF : 2 * C + (g + 1) * GF,
        ]
        in_maps.append(
            {
                "xT": np.ascontiguousarray(x[b].T),
                "wqkvT": np.ascontiguousarray(wqkv[rows].T),
                "woT": np.ascontiguousarray(wo[:, g * GF : (g + 1) * GF].T),
                "swapC": swap_mat,
            }
        )
    return in_maps


def _run(x, wqkv, wo, trace=False, trace_cores=None):
    if "nc" not in _CACHE:
        _CACHE["nc"] = _build()
    res = run_bass_kernel_spmd(
        _CACHE["nc"],
        _prep_inputs(x, wqkv, wo),
        core_ids=list(range(8)),
        trace=trace,
        trace_cores=trace_cores,
    )
    out = np.empty((B, T, C), dtype=np.float32)
    for b in range(B):
        r0, r1 = res.results[2 * b], res.results[2 * b + 1]
        o = r0["outT"].astype(np.float32) + r1["outT"].astype(np.float32)
        o[:, 3 * CHUNK :] += r0["outT2"].astype(np.float32) + r1["outT2"].astype(
            np.float32
        )
        out[b] = o.T
    return out, res


def kernel(x, wqkv, wo):
    out, _ = _run(x, wqkv, wo)
    return out

